# revision 10
# baseline (speedup 1.0000x reference)
"""RBF kernel layer (retrieval_knn): out = exp(-||x - p||^2) for x [131072, 64]
against 512 prototypes, distributed data-parallel over 8 NeuronCores.

v3: single fp16 matmul per 128-point tile + bf16 output + (optionally)
exp split across the Scalar AND Vector engines.

Math: exp(-dist2) = exp(2*S), S = lhsT.T @ rhs with
  lhsT = [x_t (64); -x_sq/2 hi; -x_sq/2 lo; ones; ones]  (fp16, per point)
  rhs  = [p_t (64); ones; ones; -p_sq/2 hi; -p_sq/2 lo]  (fp16, per proto)
K=68, one fp16 matmul per tile (1 cycle/row, 2-byte weight loads - the
f32r variant measured 667ns/matmul vs fp16's expected ~213-427ns).
fp16 feature rounding gives rel_norm ~7e-3 (gate 2e-2); the x_sq/p_sq
rows are hi/lo split so their larger magnitudes (~32) stay exact.

Points are PERMUTED within each 512-row group (tile t, partition p <->
row 4p+t) so each output-store partition writes 4KB contiguous bf16
lines instead of 4x1KB, keeping store DMA at full HBM bandwidth.

The 32 exp ACTIVATEs (2048 elems each) are the serial floor on the
Scalar engine (~63us); when DVE_ACT is enabled they alternate between
the Scalar and Vector engines (which has its own act table, loaded
manually - the framework pass only covers Scalar), cutting the
activation critical path to ~34us and leaving DMA (~53us) as the roof.
"""

import numpy as np

# Problem constants (hardcoded per harness contract; kernel.py is self-contained)
N = 131072
D = 64
M = 512
GAMMA = 1.0
NCORES = 8
NSHARD = N // NCORES  # 16384
P = 128
K1 = D + 4  # contraction: 64 x rows + 2 nxsq rows + 2 ones rows
XCHUNK = 8  # x tiles per input DMA
OCHUNK = 4  # output tiles per ACTIVATE + output DMA (PSUM 4-bank group)
DVE_ACT = False  # BIR verifier rejects InstActivation on DVE; Scalar only
DVE_GROUPS = 14  # of the 32 activation groups, how many go to DVE

_cache = {}


def _dve_activation(nc, mybir, out, in_, func, bias=0.0, scale=1.0, alpha=0.0):
    eng = nc.vector
    if isinstance(bias, float):
        bias = nc.const_aps.scalar_like(bias, in_)
    ins = [eng.lower_ap(in_)]
    for arg in (bias, scale, alpha):
        if hasattr(arg, "tensor"):
            ins.append(eng.lower_ap(arg))
        else:
            ins.append(mybir.ImmediateValue(dtype=mybir.dt.float32, value=arg))
    return eng.add_instruction(
        mybir.InstActivation(
            name=nc.get_next_instruction_name(),
            func=func,
            ins=ins,
            outs=[eng.lower_ap(out)],
        )
    )


def _build_bass(nshard=NSHARD):
    import concourse.mybir as mybir
    import concourse.tile as tile
    from concourse import bacc

    f32 = mybir.dt.float32
    f16 = mybir.dt.float16
    bf16 = mybir.dt.bfloat16
    nt = nshard // P
    ngroups = nt // OCHUNK
    assert nt % XCHUNK == 0 and XCHUNK % OCHUNK == 0

    # spread DVE-assigned groups evenly among the 32
    dve_set = {
        g for g in range(ngroups)
        if ((g + 1) * DVE_GROUPS) // ngroups > (g * DVE_GROUPS) // ngroups
    } if DVE_ACT else set()

    nc = bacc.Bacc(None, target_bir_lowering=False)
    # pre-packed on host (already column-permuted): rows 0..63 x features,
    # 64 = -x_sq/2 hi, 65 = lo, 66..67 = ones
    xp_d = nc.dram_tensor("xp", [K1, nshard], f16, kind="ExternalInput")
    # rows 0..63 p features, 64..65 = ones, 66 = -p_sq/2 hi, 67 = lo
    rhs_d = nc.dram_tensor("rhs", [K1, M], f16, kind="ExternalInput")
    out_d = nc.dram_tensor("out", [nshard, M], bf16, kind="ExternalOutput")

    with tile.TileContext(nc) as tc:
        with (
            tc.tile_pool(name="singles", bufs=1) as singles,
            tc.tile_pool(name="outp", bufs=3) as outp,
            tc.tile_pool(name="ps_o", bufs=2, space="PSUM") as ps_o,
        ):
            if dve_set:
                # exp table for the DVE (set 0 = exp_and_others); the
                # framework's table pass only covers the Scalar engine.
                nc.vector.add_instruction(
                    mybir.InstLoadActFuncSet(
                        act_func_set_id=0,
                        name=nc.get_next_instruction_name(),
                        engine=mybir.EngineType.DVE,
                        ins=[],
                        outs=[],
                    )
                )
            rhs_sb = singles.tile([K1, M], f16)
            nc.sync.dma_start(rhs_sb[:], rhs_d[:])

            # all of x stays resident in SBUF (32 KB/partition). Chunk
            # loads are issued just-in-time inside the tile loop: each
            # matmul's semaphore wait covers every DMA issued before it,
            # so front-loading all chunk DMAs would stall the first
            # matmul behind ~17 descriptor generations (~8us). Small
            # first chunks let compute start after ~4 tiles' worth.
            X_all = singles.tile([K1, nt * P], f16)
            bounds = [0, 4, 8, 16]
            while bounds[-1] < nt:
                bounds.append(min(nt, bounds[-1] + 16))
            nchunks = len(bounds) - 1
            next_chunk = 0

            for i in range(nt):
                # chunk 0 issues before tile 0; later chunks stagger one
                # tile apart so the first matmul's DMA-semaphore threshold
                # covers only rhs + chunk 0 (each matmul waits on every
                # DMA issued before it, so front-loading issues stalls
                # the pipeline start).
                while next_chunk < nchunks and max(
                    next_chunk, bounds[next_chunk] - 8
                ) <= i:
                    cs = slice(bounds[next_chunk] * P, bounds[next_chunk + 1] * P)
                    nc.sync.dma_start(X_all[:, cs], xp_d[:, cs])
                    next_chunk += 1
                k = i % OCHUNK
                g = i // OCHUNK
                if i % (2 * OCHUNK) == 0:
                    # one output tile + one store DMA per TWO activation
                    # groups: halves the store count (fewer semaphores ->
                    # shorter end-of-kernel reset chain)
                    o_sb = outp.tile([P, 2, OCHUNK, M], bf16, tag="o")
                if k == 0:
                    psum = ps_o.tile([P, OCHUNK, M], f32, tag="psum")

                nc.tensor.matmul(
                    psum[:, k, :],
                    X_all[:, i * P : (i + 1) * P],
                    rhs_sb[:],
                    start=True,
                    stop=True,
                )

                if k == OCHUNK - 1:
                    h = g % 2
                    if g in dve_set:
                        _dve_activation(
                            nc, mybir, o_sb[:, h], psum[:],
                            mybir.ActivationFunctionType.Exp,
                            bias=0.0, scale=2.0,
                        )
                    else:
                        nc.scalar.activation(
                            o_sb[:, h], psum[:],
                            mybir.ActivationFunctionType.Exp,
                            bias=0.0, scale=2.0,
                        )
                    if h == 1:
                        # partition p slot t -> row 4p+t within each
                        # 512-row half: with the host-side column
                        # permutation each partition stores two 4KB
                        # contiguous bf16 runs.
                        g0 = g - 1
                        dest = out_d[
                            g0 * OCHUNK * P : (g0 + 2) * OCHUNK * P, :
                        ].rearrange("(h p t) m -> p h t m", h=2, p=P)
                        nc.sync.dma_start(dest, o_sb[:])

    nc.finalize()
    return nc


def _get_nc():
    if "nc" not in _cache:
        _cache["nc"] = _build_bass()
    return _cache["nc"]


def _hilo16(v32):
    h = v32.astype(np.float16)
    l = (v32 - h.astype(np.float32)).astype(np.float16)
    return h, l


def _prep_core_arrays(x, prototypes, nshard):
    """Per-core host arrays: xp [68, nshard] fp16 (column-permuted), rhs
    [68, 512] fp16."""
    x = np.ascontiguousarray(np.asarray(x, dtype=np.float32))
    prototypes = np.ascontiguousarray(np.asarray(prototypes, dtype=np.float32))
    ntotal = x.shape[0]

    nxsq = (-0.5 * (x.astype(np.float64) ** 2).sum(axis=1)).astype(np.float32)
    nxh, nxl = _hilo16(nxsq)
    ones_n = np.ones(ntotal, dtype=np.float16)
    xp_full = np.concatenate(
        [x.T.astype(np.float16), nxh[None], nxl[None], ones_n[None], ones_n[None]],
        axis=0,
    )  # [68, N]

    p_sq = (prototypes.astype(np.float64) ** 2).sum(axis=1)
    nph, npl = _hilo16((-0.5 * p_sq).astype(np.float32))
    ones_m = np.ones((1, M), dtype=np.float16)
    rhs = np.ascontiguousarray(
        np.concatenate(
            [prototypes.T.astype(np.float16), ones_m, ones_m, nph[None], npl[None]],
            axis=0,
        )
    )  # [68, 512]

    # column permutation: within each 512-point block, column t*128+p
    # holds point 4p+t (so tile t partition p <-> output row 4p+t).
    blk = np.arange(OCHUNK * P).reshape(P, OCHUNK).T.ravel()  # [512]
    nblk = nshard // (OCHUNK * P)
    perm = (np.arange(nblk)[:, None] * (OCHUNK * P) + blk[None, :]).ravel()

    ncores = ntotal // nshard
    in_maps = []
    for s in range(ncores):
        cols = s * nshard + perm
        in_maps.append(
            {
                "xp": np.ascontiguousarray(xp_full[:, cols]),
                "rhs": rhs,
            }
        )
    return in_maps


def _prep_inputs(x, prototypes):
    return _prep_core_arrays(x, prototypes, NSHARD)


def _run(inputs, trace=False):
    from concourse.bass_utils import run_bass_kernel_spmd

    in_maps = _prep_inputs(inputs["x"], inputs["prototypes"])
    nc = _get_nc()
    res = run_bass_kernel_spmd(
        nc, in_maps, core_ids=list(range(NCORES)), trace=trace
    )
    out = np.concatenate(
        [np.asarray(r["out"]).astype(np.float32) for r in res.results], axis=0
    )
    return out, res


def kernel(**inputs) -> np.ndarray:
    out, _ = _run(inputs, trace=False)
    return out
